# revision 27
# baseline (speedup 1.0000x reference)
"""CLIP encoder layer (LN -> causal MHA -> residual -> LN -> GELU MLP -> residual)
as a Bass/Tile kernel for Trainium2, data-parallel over batch across 8 NeuronCores.

Layout strategy per core (one batch element):
  - matmul operands in bf16 (weights cast on the host; activations cast on the
    engine that moves them out of PSUM); all accumulation in fp32 PSUM; the
    residual stream x_h stays fp32 in SBUF. bf16 keeps the PE at 1 cycle/row
    with fast (FWL) weight loads and halves weight DMA + SBUF.
  - h kept natural [S,E] fp32 (residuals + LN stats) and transposed bf16 [E,S]
    (PE transposes; projections contract over E on partitions).
  - attention in scoresT[t_key, s_query] layout; softmax denominators come from
    a ones column appended to V (row 64 of the AV psum accumulation), kept in
    fp32(r) and inverted with the fast approximate reciprocal.
  - the additive mask is preprocessed on the HOST: per kept [128k x 512q] score
    block, transposed + clamped to -80, deduplicated into a few pattern slabs.
    All-zero (below-diagonal) blocks skip the mask add (exp reads score psum
    directly); fully-masked blocks are skipped; mask-dead leading query columns
    are skipped via live windows (floored at 256 wide). Per (head, chunk) the
    exp'd scores live in one contiguous strip so crossing blocks share one
    batched exp instruction.
  - MLP weights are loaded once; the full gelu map (bf16) stays in SBUF.
"""

import numpy as np
from contextlib import ExitStack

import ml_dtypes
import concourse.bass as bass
import concourse.mybir as mybir
import concourse.tile as tile
from concourse import bacc
from concourse.bass_utils import run_bass_kernel_spmd
from concourse.masks import make_identity

AF = mybir.ActivationFunctionType
ALU = mybir.AluOpType
F32 = mybir.dt.float32
F32R = mybir.dt.float32r
BF16 = mybir.dt.bfloat16

B, S, E, H, D, F = 8, 1024, 768, 12, 64, 3072
P = 128
ST = S // P            # 8 token tiles
ET = E // P            # 6 embed tiles
FT = F // P            # 24 ffn tiles
SC = 512               # attention s(query)-chunk
NSC = S // SC          # 2
NG = 3                 # head groups
HPG = H // NG          # 4 heads per group
GW = HPG * D           # 256 embed cols per group
MC = 256               # m2 output col chunk
SCALE = float(D) ** -0.5
EPS = 1e-5
MASK_CLAMP = -80.0     # exp(score - 80) == 0-ish for masked keys


def _r(ap):
    """Reinterpret an fp32 AP as float32r (used for the denominator path)."""
    return ap.bitcast(F32R)


def _bcast_dma(nc, dst, src_ap):
    """DMA a 1-D DRAM vector to [P, n] SBUF, broadcast along partitions."""
    bsrc = bass.AP(
        tensor=src_ap.tensor, offset=src_ap.offset, ap=[[0, P]] + list(src_ap.ap)
    )
    nc.gpsimd.dma_start(out=dst, in_=bsrc)


def build(cfg):
    """Build the Bass module.

    cfg = (blocks, n_pat); blocks is a tuple of (tk, c, kind, slot, r) with
    kind in {"clean", "cross"}; slot indexes the host-prepared maskT pattern
    slab; r is the live-window start column (floored so width >= 256).
    """
    blocks, n_pat = cfg
    nc = bacc.Bacc("TRN2", target_bir_lowering=False, debug=False, num_devices=8)

    x_t = nc.dram_tensor("x", [S, E], F32, kind="ExternalInput")
    mp_t = nc.dram_tensor("maskpat", [max(n_pat, 1), P, SC], BF16,
                          kind="ExternalInput")
    names_1d = ["bv", "bo", "b2"]
    v1 = {n: nc.dram_tensor(n, [E], F32, kind="ExternalInput") for n in names_1d}
    # [P, 60] host-packed per-channel vectors:
    # bq(6) bk(6) ln1g(6) ln1b(6) ln2g(6) ln2b(6) b1(24)
    bp_t = nc.dram_tensor("bias_pack", [P, 60], F32, kind="ExternalInput")
    # weights arrive host-pre-tiled, partition-major with long contiguous
    # per-partition lines so DMA runs at full bandwidth
    wq_t = nc.dram_tensor("wq", [P, ET * E], BF16, kind="ExternalInput")
    wk_t = nc.dram_tensor("wk", [P, ET * E], BF16, kind="ExternalInput")
    wv_t = nc.dram_tensor("wv", [P, ET * E], BF16, kind="ExternalInput")
    wo_t = nc.dram_tensor("wo", [2, P, ET * 384], BF16, kind="ExternalInput")
    w1_t = nc.dram_tensor("w1", [6, P, ET * 512], BF16, kind="ExternalInput")
    w2_t = nc.dram_tensor("w2", [E // MC, P, FT * MC], BF16,
                          kind="ExternalInput")
    out_t = nc.dram_tensor("out", [S, E], F32, kind="ExternalOutput")

    xa = x_t.ap().rearrange("(n p) e -> p n e", p=P)          # [P, ST, E]
    outa = out_t.ap().rearrange("(n p) e -> p n e", p=P)
    mpa = mp_t.ap().rearrange("n p k -> p n k")               # [P, n_pat, SC]

    binfo = {(tk, c): (kind, slot, r) for (tk, c, kind, slot, r) in blocks}
    # per chunk: clean blocks first (full width -> first AV matmul covers the
    # whole psum bank), then crossing blocks by ascending live-window r so the
    # crossing region of the exp strip is contiguous at the end.
    order = {}
    for c in range(NSC):
        ks = [(tk,) + binfo[(tk, c)] for tk in range(ST) if (tk, c) in binfo]
        order[c] = sorted(ks, key=lambda t: (t[1] != "clean", t[3], t[0]))

    with tile.TileContext(nc) as tc, ExitStack() as top:
        persist = top.enter_context(tc.tile_pool(name="persist", bufs=1))
        psum = top.enter_context(tc.tile_pool(name="psum", bufs=1, space="PSUM"))

        x_hs = [persist.tile([P, E], F32, name=f"x_h{i}") for i in range(ST)]
        identity = persist.tile([P, P], BF16, name="identity")
        make_identity(nc, identity)
        small = {"eps": persist.tile([P, 1], F32, name="eps_t")}
        nc.vector.memset(small["eps"], EPS)
        ones64 = persist.tile([1, 64], F32, name="ones64")
        nc.vector.memset(ones64, 1.0)
        bpk = persist.tile([P, 60], F32, name="bpk")
        bo_b = persist.tile([P, E], F32, name="bo_b")
        b2b = persist.tile([P, E], F32, name="b2b")
        bqs, bks = bpk[:, 0:6], bpk[:, 6:12]
        g1c, b1cc = bpk[:, 12:18], bpk[:, 18:24]
        g2c, b2cc = bpk[:, 24:30], bpk[:, 30:36]
        b1c = bpk[:, 36:60]

        nc.gpsimd.dma_start(bpk, bp_t.ap())
        for i in range(ST):
            nc.gpsimd.dma_start(x_hs[i], xa[:, i, :])

        h1T = [persist.tile([P, ET, SC], BF16, tag="hT", bufs=2, name="h1T")
               for _ in range(NSC)]

        def layernorm(pool, x_slice, out_tmp, seng=None):
            seng = seng or nc.vector
            xr = x_slice.rearrange("p (n s) -> p n s", s=256)
            stats = pool.tile([P, 3, 6], F32, tag="lnstats", bufs=4, name="st")
            for sg in range(3):
                seng.bn_stats(out=stats[:, sg, :], in_=xr[:, sg, :])
            mv = pool.tile([P, 2], F32, tag="lnmv", bufs=4, name="mv")
            seng.bn_aggr(out=mv, in_=stats)
            std = pool.tile([P, 2], F32, tag="lnrstd", bufs=4, name="std")
            nc.scalar.activation(
                out=std[:, 0:1], in_=mv[:, 1:2], func=AF.Sqrt,
                bias=small["eps"], scale=1.0
            )
            nc.vector.reciprocal_approx_fast(out=std[:, 1:2], in_=std[:, 0:1])
            nc.vector.tensor_scalar(
                out=out_tmp,
                in0=x_slice,
                scalar1=mv[:, 0:1],
                scalar2=std[:, 1:2],
                op0=ALU.subtract,
                op1=ALU.mult,
            )

        def ln_tile(pool, i, dstT, gc, bc, on_act=True, seng=None):
            htmp = pool.tile([P, E], BF16, tag="lntmp", bufs=3, name="htmp")
            layernorm(pool, x_hs[i], htmp, seng=seng)
            for j in range(ET):
                pt = psum.tile([P, P], BF16, tag="tp", bufs=2, name="pt")
                nc.tensor.transpose(pt, htmp[:, j * P : (j + 1) * P], identity)
                dst = dstT[i // 4][:, j, (i % 4) * P : (i % 4 + 1) * P]
                if on_act:
                    nc.scalar.activation(
                        out=dst,
                        in_=pt,
                        func=AF.Identity,
                        bias=bc[:, j : j + 1],
                        scale=gc[:, j : j + 1],
                    )
                else:
                    nc.vector.tensor_scalar(
                        out=dst,
                        in0=pt,
                        scalar1=gc[:, j : j + 1],
                        scalar2=bc[:, j : j + 1],
                        op0=ALU.mult,
                        op1=ALU.add,
                    )

        with tc.tile_pool(name="attnp", bufs=1) as ap_:
            maskpat = ap_.tile([P, max(n_pat, 1), SC], BF16, name="maskpat")
            bv_b = ap_.tile([P, E], F32, name="bv_b")
            OT = ap_.tile([P, ET, S], BF16, name="OT")

            wall = {}
            for nm, t in (("q", wq_t), ("k", wk_t), ("v", wv_t)):
                w = ap_.tile([P, ET, E], BF16, tag="wsl", bufs=3, name="w_all")
                nc.sync.dma_start(w, t.ap().rearrange("p (k m) -> p k m", m=E))
                wall[nm] = w
            wsl = {
                g: tuple(
                    wall[nm][:, :, g * GW : (g + 1) * GW] for nm in "qkv"
                )
                for g in range(NG)
            }
            wo_c = [
                ap_.tile([P, ET, 384], BF16, tag="wo", bufs=2, name="wo_c")
                for _ in range(2)
            ]

            def qk_proj(g, c):
                wq_g, wk_g, _ = wsl[g]
                qT_g, kT_g = qkT[g]
                for w_g, dstT, bias, scl in (
                    (wq_g, qT_g, bqs, SCALE),
                    (wk_g, kT_g, bks, None),
                ):
                    for jl in range(2):
                        jj = 2 * g + jl
                        ps = psum.tile([P, SC], F32, tag="pj", bufs=2, name="psq")
                        for ek in range(ET):
                            nc.tensor.matmul(
                                ps,
                                w_g[:, ek, jl * P : (jl + 1) * P],
                                h1T[c][:, ek, :],
                                start=(ek == 0),
                                stop=(ek == ET - 1),
                            )
                        if scl is None:
                            nc.vector.tensor_scalar(
                                out=dstT[:, jl, c * SC : (c + 1) * SC],
                                in0=ps,
                                scalar1=bias[:, jj : jj + 1],
                                scalar2=None,
                                op0=ALU.add,
                            )
                        else:
                            nc.vector.tensor_scalar(
                                out=dstT[:, jl, c * SC : (c + 1) * SC],
                                in0=ps,
                                scalar1=bias[:, jj : jj + 1],
                                scalar2=scl,
                                op0=ALU.add,
                                op1=ALU.mult,
                            )

            # LN1 interleaved with group-0 q/k projections
            qkT = {
                g: (
                    ap_.tile([P, 2, S], BF16, tag="qT", bufs=2, name="qT_g"),
                    ap_.tile([P, 2, S], BF16, tag="kT", bufs=2, name="kT_g"),
                )
                for g in range(NG)
            }
            for i in range(4):
                ln_tile(ap_, i, h1T, g1c, b1cc)
            qk_proj(0, 0)
            for i in range(4, ST):
                ln_tile(ap_, i, h1T, g1c, b1cc)
            qk_proj(0, 1)
            # deferred gpsimd-ring DMAs (keeps LN1's ring barrier short)
            nc.gpsimd.dma_start(maskpat, mpa)
            _bcast_dma(nc, bv_b, v1["bv"].ap())
            _bcast_dma(nc, bo_b, v1["bo"].ap())
            _bcast_dma(nc, b2b, v1["b2"].ap())
            for c2 in range(2):
                nc.gpsimd.dma_start(
                    wo_c[c2], wo_t.ap()[c2].rearrange("p (k m) -> p k m", m=384)
                )
            for i in range(ST):  # h += bo early (runs on idle gpsimd)
                nc.gpsimd.tensor_add(x_hs[i], x_hs[i], bo_b)

            for g in range(NG):
                _, _, wv_g = wsl[g]
                qT_g, kT_g = qkT[g]
                vaug = ap_.tile(
                    [P, ST, HPG, D + 1], BF16, tag="vg", bufs=3, name="vaug"
                )
                nc.gpsimd.memset(vaug[:, :, :, D : D + 1], 1.0)
                bvr = bv_b[:, g * GW : (g + 1) * GW].rearrange(
                    "p (h d) -> p h d", d=D
                )
                for i in range(ST):
                    ps = psum.tile([P, SC], F32, tag="pj", bufs=2, name="psv")
                    for ek in range(ET):
                        nc.tensor.matmul(
                            ps[:, :GW],
                            h1T[i // 4][:, ek, (i % 4) * P : (i % 4 + 1) * P],
                            wv_g[:, ek, :],
                            start=(ek == 0),
                            stop=(ek == ET - 1),
                        )
                    nc.vector.tensor_tensor(
                        out=vaug[:, i, :, 0:D],
                        in0=ps[:, :GW].rearrange("p (h d) -> p h d", d=D),
                        in1=bvr,
                        op=ALU.add,
                    )

                # per-head attention: emit both chunks' scores/exp first,
                # then both AV/normalize passes (keeps PE fed while ACT exps);
                # halfway through, emit the next group's q/k projections so the
                # PE has dense work while ACT/DVE catch up
                for hl in range(HPG):
                    if hl == 2 and g + 1 < NG:
                        qk_proj(g + 1, 0)
                        qk_proj(g + 1, 1)
                    hg = g * HPG + hl
                    jl, roff = hl // 2, 64 * (hl % 2)
                    st_c = {}
                    for c in range(NSC):
                        blks = order[c]
                        nb = len(blks)
                        strip = ap_.tile([P, nb * SC], BF16, tag="exs", bufs=5,
                                         name="strip")
                        st_c[c] = strip
                        ncl = sum(1 for t in blks if t[1] == "clean")
                        for n, (tk, kind, slot, r) in enumerate(blks):
                            pss = psum.tile([P, SC], F32, tag="sc", bufs=2,
                                            name="pss")
                            nc.tensor.matmul(
                                pss[:, r:],
                                kT_g[roff : roff + 64, jl,
                                     tk * P : (tk + 1) * P],
                                qT_g[roff : roff + 64, jl,
                                     c * SC + r : (c + 1) * SC],
                                start=True,
                                stop=True,
                            )
                            sl = strip[:, n * SC + r : (n + 1) * SC]
                            if kind == "cross":
                                nc.vector.tensor_tensor(
                                    out=sl, in0=pss[:, r:],
                                    in1=maskpat[:, slot, r:], op=ALU.add,
                                )
                            else:
                                nc.scalar.activation(sl, pss[:, r:],
                                                     func=AF.Exp)
                        if ncl < nb:
                            reg = strip[:, ncl * SC : nb * SC]
                            nc.scalar.activation(reg, reg, func=AF.Exp)
                    for c in range(NSC):
                        blks = order[c]
                        nb = len(blks)
                        strip = st_c[c]
                        psa = psum.tile([P, SC], F32, tag="av", bufs=2,
                                        name="psa")
                        for n, (tk, kind, slot, r) in enumerate(blks):
                            nc.tensor.matmul(
                                psa[: D + 1, r:],
                                vaug[:, tk, hl, :],
                                strip[:, n * SC + r : (n + 1) * SC],
                                start=(n == 0),
                                stop=(n == nb - 1),
                            )
                        denr = ap_.tile([1, SC], F32, tag="denr", bufs=4,
                                        name="denr")
                        nc.scalar.copy(_r(denr), psa[D : D + 1, :])
                        psb = psum.tile([P, SC], F32, tag="tp", bufs=2,
                                        name="psb")
                        nc.tensor.matmul(
                            psb[:64, :], _r(ones64), _r(denr), start=True,
                            stop=True
                        )
                        bcs = ap_.tile([64, SC], F32, tag="bcs", bufs=4,
                                       name="bcs")
                        nc.vector.reciprocal_approx_fast(out=bcs,
                                                         in_=psb[:64, :])
                        ro2 = 64 * (hg % 2)
                        nc.vector.tensor_tensor(
                            out=OT[ro2 : ro2 + 64, hg // 2,
                                   c * SC : (c + 1) * SC],
                            in0=psa[:D, :],
                            in1=bcs,
                            op=ALU.mult,
                        )

            # ---------------- output projection + residual + LN2 -------------
            h2T = [persist.tile([P, ET, SC], BF16, tag="hT", bufs=2,
                                name="h2T") for _ in range(NSC)]
            for i in range(ST):
                for c2 in range(2):
                    cs = slice(c2 * 384, (c2 + 1) * 384)
                    ps = psum.tile([P, SC], F32, tag="pj", bufs=2, name="pso")
                    for ek in range(ET):
                        nc.tensor.matmul(
                            ps[:, :384],
                            OT[:, ek, i * P : (i + 1) * P],
                            wo_c[c2][:, ek, :],
                            start=(ek == 0),
                            stop=(ek == ET - 1),
                        )
                    nc.vector.tensor_tensor(out=x_hs[i][:, cs], in0=ps[:, :384],
                                            in1=x_hs[i][:, cs], op=ALU.add)
                ln_tile(ap_, i, h2T, g2c, b2cc)

        # ---------------- MLP + final residual (weights loaded once) ---------
        with tc.tile_pool(name="mlpp", bufs=1) as mp:
            m1g = mp.tile([P, FT, S], BF16, name="m1g")

            def m1_chunk(w1_sb, w, ftl, sc):
                ft = 4 * w + ftl
                ps = psum.tile([P, SC], F32, tag="pj", bufs=2, name="psm1")
                for ek in range(ET):
                    nc.tensor.matmul(
                        ps,
                        w1_sb[:, ek, ftl * P : (ftl + 1) * P],
                        h2T[sc][:, ek, :],
                        start=(ek == 0),
                        stop=(ek == ET - 1),
                    )
                nc.scalar.activation(
                    out=m1g[:, ft, sc * SC : (sc + 1) * SC],
                    in_=ps,
                    func=AF.Gelu,
                    bias=b1c[:, ft : ft + 1],
                    scale=1.0,
                )

            for i in range(ST):
                nc.gpsimd.tensor_add(x_hs[i], x_hs[i], b2b)
            # prefetch all w2 chunks early on the gpsimd queue
            w2_cs = []
            for c2 in range(E // MC):
                w2_c = mp.tile([P, FT, MC], BF16, tag="w2", bufs=3, name="w2_c")
                nc.gpsimd.dma_start(
                    w2_c, w2_t.ap()[c2].rearrange("p (k m) -> p k m", m=MC)
                )
                w2_cs.append(w2_c)
            for w in range(6):  # w1 column groups of 4 f-tiles
                w1_sb = mp.tile([P, ET, 512], BF16, tag="w1s", bufs=3,
                                name="w1_sb")
                nc.sync.dma_start(
                    w1_sb, w1_t.ap()[w].rearrange("p (k m) -> p k m", m=512)
                )
                for ftl in range(4):
                    for sc in range(NSC):
                        m1_chunk(w1_sb, w, ftl, sc)
            for c2 in range(E // MC):
                cs = slice(c2 * MC, (c2 + 1) * MC)
                for i in range(ST):
                    ps2 = psum.tile([P, SC], F32, tag="sc", bufs=2, name="psm2")
                    for fk in range(FT):
                        nc.tensor.matmul(
                            ps2[:, :MC],
                            m1g[:, fk, i * P : (i + 1) * P],
                            w2_cs[c2][:, fk, :],
                            start=(fk == 0),
                            stop=(fk == FT - 1),
                        )
                    otile = mp.tile([P, MC], F32, tag="otile", bufs=4,
                                    name="otile")
                    nc.vector.tensor_tensor(out=otile, in0=ps2[:, :MC],
                                            in1=x_hs[i][:, cs], op=ALU.add)
                    nc.sync.dma_start(outa[:, i, cs], otile)

    nc.compile()
    return nc


_CACHE = {}


def _get_nc(cfg):
    if cfg not in _CACHE:
        _CACHE[cfg] = build(cfg)
    return _CACHE[cfg]


def _prepare(inputs):
    """Host-side prep: classify score blocks, build mask pattern slabs, cast
    weights to bf16, and return (nc, in_maps) for run_bass_kernel_spmd."""
    inp = {
        k: np.ascontiguousarray(np.asarray(v, np.float32))
        for k, v in inputs.items()
    }
    mask = inp["mask"]  # [B, 1, S, S]

    blocks = []
    pat_key_to_slot = {}
    pats = []  # list of [B, P, SC] arrays
    for c in range(NSC):
        for tk in range(ST):
            blk = mask[:, 0, c * SC : (c + 1) * SC, tk * P : (tk + 1) * P]
            live = blk > -1e8  # [B, SC, P]
            if not live.any():
                continue  # fully masked for every batch
            if (blk == 0).all():
                blocks.append((tk, c, "clean", 0, 0))
                continue
            qlive = int(np.argmax(live.any(axis=(0, 2))))
            r = min(qlive, SC - 256)
            patT = np.maximum(blk, MASK_CLAMP).transpose(0, 2, 1)  # [B, P, SC]
            key = patT.tobytes()
            if key not in pat_key_to_slot:
                pat_key_to_slot[key] = len(pats)
                pats.append(patT)
            blocks.append((tk, c, "cross", pat_key_to_slot[key], r))
    n_pat = len(pats)
    if n_pat:
        maskpat = np.ascontiguousarray(
            np.stack(pats, axis=1).astype(ml_dtypes.bfloat16)
        )  # [B, n_pat, P, SC]
    else:
        maskpat = np.zeros((B, 1, P, SC), ml_dtypes.bfloat16)

    cfg = (tuple(blocks), n_pat)
    nc = _get_nc(cfg)

    shared = {k: inp[k] for k in ["bv", "bo", "b2"]}
    pcol = lambda v, n: inp[v].reshape(n, P).T
    shared["bias_pack"] = np.ascontiguousarray(
        np.concatenate(
            [pcol("bq", 6), pcol("bk", 6), pcol("ln1_g", 6), pcol("ln1_b", 6),
             pcol("ln2_g", 6), pcol("ln2_b", 6), pcol("b1", 24)],
            axis=1,
        ),
        np.float32,
    )
    # pre-tile weights (partition-major, chunk-contiguous) and cast to bf16 so
    # every DMA reads multi-KB contiguous lines per partition
    bf = lambda a: np.ascontiguousarray(a.astype(ml_dtypes.bfloat16))
    shared["wq"] = bf(inp["wq"].reshape(ET, P, E).transpose(1, 0, 2)
                      .reshape(P, ET * E))
    shared["wk"] = bf(inp["wk"].reshape(ET, P, E).transpose(1, 0, 2)
                      .reshape(P, ET * E))
    shared["wv"] = bf(inp["wv"].reshape(ET, P, E).transpose(1, 0, 2)
                      .reshape(P, ET * E))
    shared["wo"] = bf(inp["wo"].reshape(ET, P, 2, 384).transpose(2, 1, 0, 3)
                      .reshape(2, P, ET * 384))
    shared["w1"] = bf(inp["w1"].reshape(ET, P, 6, 512).transpose(2, 1, 0, 3)
                      .reshape(6, P, ET * 512))
    shared["w2"] = bf(inp["w2"].reshape(FT, P, E // MC, MC)
                      .transpose(2, 1, 0, 3).reshape(E // MC, P, FT * MC))
    in_maps = [
        {"x": inp["x"][b], "maskpat": maskpat[b], **shared} for b in range(B)
    ]
    return nc, in_maps


def kernel(**inputs) -> np.ndarray:
    nc, in_maps = _prepare(inputs)
    res = run_bass_kernel_spmd(nc, in_maps, core_ids=list(range(B)))
    return np.stack([res.results[b]["out"] for b in range(B)], axis=0)


if __name__ == "__main__":
    # smoke build with the causal block pattern
    blocks = []
    for c in range(NSC):
        for tk in range(ST):
            lo, hi = tk * P, (tk + 1) * P - 1  # key range
            qlo, qhi = c * SC, (c + 1) * SC - 1
            if lo > qhi:
                continue  # fully masked
            if hi <= qlo:
                blocks.append((tk, c, "clean", 0, 0))
            else:
                r = min(max(0, lo - qlo), SC - 256)
                blocks.append((tk, c, "cross", (lo - qlo) // P, r))
    build((tuple(blocks), 4))
    print("build ok")


# revision 28
# speedup vs baseline: 1.0033x; 1.0033x over previous
"""CLIP encoder layer (LN -> causal MHA -> residual -> LN -> GELU MLP -> residual)
as a Bass/Tile kernel for Trainium2, data-parallel over batch across 8 NeuronCores.

Layout strategy per core (one batch element):
  - matmul operands in bf16 (weights cast on the host; activations cast on the
    engine that moves them out of PSUM); all accumulation in fp32 PSUM; the
    residual stream x_h stays fp32 in SBUF. bf16 keeps the PE at 1 cycle/row
    with fast (FWL) weight loads and halves weight DMA + SBUF.
  - h kept natural [S,E] fp32 (residuals + LN stats) and transposed bf16 [E,S]
    (PE transposes; projections contract over E on partitions).
  - attention in scoresT[t_key, s_query] layout; softmax denominators come from
    a ones column appended to V (row 64 of the AV psum accumulation), kept in
    fp32(r) and inverted with the fast approximate reciprocal.
  - the additive mask is preprocessed on the HOST: per kept [128k x 512q] score
    block, transposed + clamped to -80, deduplicated into a few pattern slabs.
    All-zero (below-diagonal) blocks skip the mask add (exp reads score psum
    directly); fully-masked blocks are skipped; mask-dead leading query columns
    are skipped via live windows (floored at 256 wide). Per (head, chunk) the
    exp'd scores live in one contiguous strip so crossing blocks share one
    batched exp instruction.
  - MLP weights are loaded once; the full gelu map (bf16) stays in SBUF.
"""

import numpy as np
from contextlib import ExitStack

import ml_dtypes
import concourse.bass as bass
import concourse.mybir as mybir
import concourse.tile as tile
from concourse import bacc
from concourse.bass_utils import run_bass_kernel_spmd
from concourse.masks import make_identity

AF = mybir.ActivationFunctionType
ALU = mybir.AluOpType
F32 = mybir.dt.float32
F32R = mybir.dt.float32r
BF16 = mybir.dt.bfloat16

B, S, E, H, D, F = 8, 1024, 768, 12, 64, 3072
P = 128
ST = S // P            # 8 token tiles
ET = E // P            # 6 embed tiles
FT = F // P            # 24 ffn tiles
SC = 512               # attention s(query)-chunk
NSC = S // SC          # 2
NG = 3                 # head groups
HPG = H // NG          # 4 heads per group
GW = HPG * D           # 256 embed cols per group
MC = 256               # m2 output col chunk
SCALE = float(D) ** -0.5
EPS = 1e-5
MASK_CLAMP = -80.0     # exp(score - 80) == 0-ish for masked keys


def _r(ap):
    """Reinterpret an fp32 AP as float32r (used for the denominator path)."""
    return ap.bitcast(F32R)


def _bcast_dma(nc, dst, src_ap):
    """DMA a 1-D DRAM vector to [P, n] SBUF, broadcast along partitions."""
    bsrc = bass.AP(
        tensor=src_ap.tensor, offset=src_ap.offset, ap=[[0, P]] + list(src_ap.ap)
    )
    nc.gpsimd.dma_start(out=dst, in_=bsrc)


def build(cfg):
    """Build the Bass module.

    cfg = (blocks, n_pat); blocks is a tuple of (tk, c, kind, slot, r) with
    kind in {"clean", "cross"}; slot indexes the host-prepared maskT pattern
    slab; r is the live-window start column (floored so width >= 256).
    """
    blocks, n_pat = cfg
    nc = bacc.Bacc("TRN2", target_bir_lowering=False, debug=False, num_devices=8)

    x_t = nc.dram_tensor("x", [S, E], F32, kind="ExternalInput")
    mp_t = nc.dram_tensor("maskpat", [max(n_pat, 1), P, SC], BF16,
                          kind="ExternalInput")
    names_1d = ["bv", "bo", "b2"]
    v1 = {n: nc.dram_tensor(n, [E], F32, kind="ExternalInput") for n in names_1d}
    # [P, 60] host-packed per-channel vectors:
    # bq(6) bk(6) ln1g(6) ln1b(6) ln2g(6) ln2b(6) b1(24)
    bp_t = nc.dram_tensor("bias_pack", [P, 60], F32, kind="ExternalInput")
    # weights arrive host-pre-tiled, partition-major with long contiguous
    # per-partition lines so DMA runs at full bandwidth
    wq_t = nc.dram_tensor("wq", [P, ET * E], BF16, kind="ExternalInput")
    wk_t = nc.dram_tensor("wk", [P, ET * E], BF16, kind="ExternalInput")
    wv_t = nc.dram_tensor("wv", [P, ET * E], BF16, kind="ExternalInput")
    wo_t = nc.dram_tensor("wo", [2, P, ET * 384], BF16, kind="ExternalInput")
    w1_t = nc.dram_tensor("w1", [6, P, ET * 512], BF16, kind="ExternalInput")
    w2_t = nc.dram_tensor("w2", [E // MC, P, FT * MC], BF16,
                          kind="ExternalInput")
    out_t = nc.dram_tensor("out", [S, E], F32, kind="ExternalOutput")

    xa = x_t.ap().rearrange("(n p) e -> p n e", p=P)          # [P, ST, E]
    outa = out_t.ap().rearrange("(n p) e -> p n e", p=P)
    mpa = mp_t.ap().rearrange("n p k -> p n k")               # [P, n_pat, SC]

    binfo = {(tk, c): (kind, slot, r) for (tk, c, kind, slot, r) in blocks}
    # per chunk: clean blocks first (full width -> first AV matmul covers the
    # whole psum bank), then crossing blocks by ascending live-window r so the
    # crossing region of the exp strip is contiguous at the end.
    order = {}
    for c in range(NSC):
        ks = [(tk,) + binfo[(tk, c)] for tk in range(ST) if (tk, c) in binfo]
        order[c] = sorted(ks, key=lambda t: (t[1] != "clean", t[3], t[0]))

    with tile.TileContext(nc) as tc, ExitStack() as top:
        persist = top.enter_context(tc.tile_pool(name="persist", bufs=1))
        psum = top.enter_context(tc.tile_pool(name="psum", bufs=1, space="PSUM"))

        x_hs = [persist.tile([P, E], F32, name=f"x_h{i}") for i in range(ST)]
        identity = persist.tile([P, P], BF16, name="identity")
        make_identity(nc, identity)
        small = {"eps": persist.tile([P, 1], F32, name="eps_t")}
        nc.vector.memset(small["eps"], EPS)
        ones64 = persist.tile([1, 64], F32, name="ones64")
        nc.vector.memset(ones64, 1.0)
        bpk = persist.tile([P, 60], F32, name="bpk")
        bo_b = persist.tile([P, E], F32, name="bo_b")
        b2b = persist.tile([P, E], F32, name="b2b")
        bqs, bks = bpk[:, 0:6], bpk[:, 6:12]
        g1c, b1cc = bpk[:, 12:18], bpk[:, 18:24]
        g2c, b2cc = bpk[:, 24:30], bpk[:, 30:36]
        b1c = bpk[:, 36:60]

        nc.gpsimd.dma_start(bpk, bp_t.ap())
        for i in range(ST):
            nc.gpsimd.dma_start(x_hs[i], xa[:, i, :])

        h1T = [persist.tile([P, ET, SC], BF16, tag="hT", bufs=2, name="h1T")
               for _ in range(NSC)]

        def layernorm(pool, x_slice, out_tmp, seng=None):
            seng = seng or nc.vector
            xr = x_slice.rearrange("p (n s) -> p n s", s=256)
            stats = pool.tile([P, 3, 6], F32, tag="lnstats", bufs=4, name="st")
            for sg in range(3):
                seng.bn_stats(out=stats[:, sg, :], in_=xr[:, sg, :])
            mv = pool.tile([P, 2], F32, tag="lnmv", bufs=4, name="mv")
            seng.bn_aggr(out=mv, in_=stats)
            std = pool.tile([P, 2], F32, tag="lnrstd", bufs=4, name="std")
            nc.scalar.activation(
                out=std[:, 0:1], in_=mv[:, 1:2], func=AF.Sqrt,
                bias=small["eps"], scale=1.0
            )
            nc.vector.reciprocal_approx_fast(out=std[:, 1:2], in_=std[:, 0:1])
            nc.vector.tensor_scalar(
                out=out_tmp,
                in0=x_slice,
                scalar1=mv[:, 0:1],
                scalar2=std[:, 1:2],
                op0=ALU.subtract,
                op1=ALU.mult,
            )

        def ln_tile(pool, i, dstT, gc, bc, on_act=True, seng=None):
            htmp = pool.tile([P, E], BF16, tag="lntmp", bufs=3, name="htmp")
            layernorm(pool, x_hs[i], htmp, seng=seng)
            for j in range(ET):
                pt = psum.tile([P, P], BF16, tag="tp", bufs=2, name="pt")
                nc.tensor.transpose(pt, htmp[:, j * P : (j + 1) * P], identity)
                dst = dstT[i // 4][:, j, (i % 4) * P : (i % 4 + 1) * P]
                if on_act:
                    nc.scalar.activation(
                        out=dst,
                        in_=pt,
                        func=AF.Identity,
                        bias=bc[:, j : j + 1],
                        scale=gc[:, j : j + 1],
                    )
                else:
                    nc.vector.tensor_scalar(
                        out=dst,
                        in0=pt,
                        scalar1=gc[:, j : j + 1],
                        scalar2=bc[:, j : j + 1],
                        op0=ALU.mult,
                        op1=ALU.add,
                    )

        with tc.tile_pool(name="attnp", bufs=1) as ap_:
            maskpat = ap_.tile([P, max(n_pat, 1), SC], BF16, name="maskpat")
            bv_b = ap_.tile([P, E], F32, name="bv_b")
            OT = ap_.tile([P, ET, S], BF16, name="OT")

            wall = {}
            for nm, t in (("q", wq_t), ("k", wk_t), ("v", wv_t)):
                w = ap_.tile([P, ET, E], BF16, tag="wsl", bufs=3, name="w_all")
                nc.sync.dma_start(w, t.ap().rearrange("p (k m) -> p k m", m=E))
                wall[nm] = w
            wsl = {
                g: tuple(
                    wall[nm][:, :, g * GW : (g + 1) * GW] for nm in "qkv"
                )
                for g in range(NG)
            }
            wo_c = [
                ap_.tile([P, ET, 384], BF16, tag="wo", bufs=2, name="wo_c")
                for _ in range(2)
            ]

            def qk_proj(g, c):
                wq_g, wk_g, _ = wsl[g]
                qT_g, kT_g = qkT[g]
                for w_g, dstT, bias, scl in (
                    (wq_g, qT_g, bqs, SCALE),
                    (wk_g, kT_g, bks, None),
                ):
                    for jl in range(2):
                        jj = 2 * g + jl
                        ps = psum.tile([P, SC], F32, tag="pj", bufs=2, name="psq")
                        for ek in range(ET):
                            nc.tensor.matmul(
                                ps,
                                w_g[:, ek, jl * P : (jl + 1) * P],
                                h1T[c][:, ek, :],
                                start=(ek == 0),
                                stop=(ek == ET - 1),
                            )
                        if scl is None:
                            nc.vector.tensor_scalar(
                                out=dstT[:, jl, c * SC : (c + 1) * SC],
                                in0=ps,
                                scalar1=bias[:, jj : jj + 1],
                                scalar2=None,
                                op0=ALU.add,
                            )
                        else:
                            nc.vector.tensor_scalar(
                                out=dstT[:, jl, c * SC : (c + 1) * SC],
                                in0=ps,
                                scalar1=bias[:, jj : jj + 1],
                                scalar2=scl,
                                op0=ALU.add,
                                op1=ALU.mult,
                            )

            # LN1 interleaved with group-0 q/k projections
            qkT = {
                g: (
                    ap_.tile([P, 2, S], BF16, tag="qT", bufs=2, name="qT_g"),
                    ap_.tile([P, 2, S], BF16, tag="kT", bufs=2, name="kT_g"),
                )
                for g in range(NG)
            }
            for i in range(4):
                ln_tile(ap_, i, h1T, g1c, b1cc)
            qk_proj(0, 0)
            for i in range(4, ST):
                ln_tile(ap_, i, h1T, g1c, b1cc)
            qk_proj(0, 1)
            # deferred gpsimd-ring DMAs (keeps LN1's ring barrier short)
            nc.gpsimd.dma_start(maskpat, mpa)
            _bcast_dma(nc, bv_b, v1["bv"].ap())
            _bcast_dma(nc, bo_b, v1["bo"].ap())
            _bcast_dma(nc, b2b, v1["b2"].ap())
            for c2 in range(2):
                nc.gpsimd.dma_start(
                    wo_c[c2], wo_t.ap()[c2].rearrange("p (k m) -> p k m", m=384)
                )
            for i in range(ST):  # h += bo early (runs on idle gpsimd)
                nc.gpsimd.tensor_add(x_hs[i], x_hs[i], bo_b)

            for g in range(NG):
                _, _, wv_g = wsl[g]
                qT_g, kT_g = qkT[g]
                vaug = ap_.tile(
                    [P, ST, HPG, D + 1], BF16, tag="vg", bufs=2, name="vaug"
                )
                nc.gpsimd.memset(vaug[:, :, :, D : D + 1], 1.0)
                bvr = bv_b[:, g * GW : (g + 1) * GW].rearrange(
                    "p (h d) -> p h d", d=D
                )
                for i in range(ST):
                    ps = psum.tile([P, SC], F32, tag="pj", bufs=2, name="psv")
                    for ek in range(ET):
                        nc.tensor.matmul(
                            ps[:, :GW],
                            h1T[i // 4][:, ek, (i % 4) * P : (i % 4 + 1) * P],
                            wv_g[:, ek, :],
                            start=(ek == 0),
                            stop=(ek == ET - 1),
                        )
                    nc.vector.tensor_tensor(
                        out=vaug[:, i, :, 0:D],
                        in0=ps[:, :GW].rearrange("p (h d) -> p h d", d=D),
                        in1=bvr,
                        op=ALU.add,
                    )

                # per-head attention: emit both chunks' scores/exp first,
                # then both AV/normalize passes (keeps PE fed while ACT exps);
                # halfway through, emit the next group's q/k projections so the
                # PE has dense work while ACT/DVE catch up
                for hl in range(HPG):
                    if hl == 2 and g + 1 < NG:
                        qk_proj(g + 1, 0)
                        qk_proj(g + 1, 1)
                    hg = g * HPG + hl
                    jl, roff = hl // 2, 64 * (hl % 2)
                    st_c = {}
                    for c in range(NSC):
                        blks = order[c]
                        nb = len(blks)
                        strip = ap_.tile([P, nb * SC], BF16, tag="exs", bufs=4,
                                         name="strip")
                        st_c[c] = strip
                        ncl = sum(1 for t in blks if t[1] == "clean")
                        for n, (tk, kind, slot, r) in enumerate(blks):
                            pss = psum.tile([P, SC], F32, tag="sc", bufs=2,
                                            name="pss")
                            nc.tensor.matmul(
                                pss[:, r:],
                                kT_g[roff : roff + 64, jl,
                                     tk * P : (tk + 1) * P],
                                qT_g[roff : roff + 64, jl,
                                     c * SC + r : (c + 1) * SC],
                                start=True,
                                stop=True,
                            )
                            sl = strip[:, n * SC + r : (n + 1) * SC]
                            if kind == "cross":
                                nc.vector.tensor_tensor(
                                    out=sl, in0=pss[:, r:],
                                    in1=maskpat[:, slot, r:], op=ALU.add,
                                )
                            else:
                                nc.scalar.activation(sl, pss[:, r:],
                                                     func=AF.Exp)
                        if ncl < nb:
                            reg = strip[:, ncl * SC : nb * SC]
                            nc.scalar.activation(reg, reg, func=AF.Exp)
                    for c in range(NSC):
                        blks = order[c]
                        nb = len(blks)
                        strip = st_c[c]
                        psa = psum.tile([P, SC], F32, tag="av", bufs=2,
                                        name="psa")
                        for n, (tk, kind, slot, r) in enumerate(blks):
                            nc.tensor.matmul(
                                psa[: D + 1, r:],
                                vaug[:, tk, hl, :],
                                strip[:, n * SC + r : (n + 1) * SC],
                                start=(n == 0),
                                stop=(n == nb - 1),
                            )
                        denr = ap_.tile([1, SC], F32, tag="denr", bufs=2,
                                        name="denr")
                        nc.vector.tensor_copy(_r(denr), psa[D : D + 1, :])
                        psb = psum.tile([P, SC], F32, tag="tp", bufs=2,
                                        name="psb")
                        nc.tensor.matmul(
                            psb[:64, :], _r(ones64), _r(denr), start=True,
                            stop=True
                        )
                        bcs = ap_.tile([64, SC], F32, tag="bcs", bufs=2,
                                       name="bcs")
                        nc.vector.reciprocal_approx_fast(out=bcs,
                                                         in_=psb[:64, :])
                        ro2 = 64 * (hg % 2)
                        nc.vector.tensor_tensor(
                            out=OT[ro2 : ro2 + 64, hg // 2,
                                   c * SC : (c + 1) * SC],
                            in0=psa[:D, :],
                            in1=bcs,
                            op=ALU.mult,
                        )

            # ---------------- output projection + residual + LN2 -------------
            h2T = [persist.tile([P, ET, SC], BF16, tag="hT", bufs=2,
                                name="h2T") for _ in range(NSC)]
            for i in range(ST):
                for c2 in range(2):
                    cs = slice(c2 * 384, (c2 + 1) * 384)
                    ps = psum.tile([P, SC], F32, tag="pj", bufs=2, name="pso")
                    for ek in range(ET):
                        nc.tensor.matmul(
                            ps[:, :384],
                            OT[:, ek, i * P : (i + 1) * P],
                            wo_c[c2][:, ek, :],
                            start=(ek == 0),
                            stop=(ek == ET - 1),
                        )
                    nc.vector.tensor_tensor(out=x_hs[i][:, cs], in0=ps[:, :384],
                                            in1=x_hs[i][:, cs], op=ALU.add)
                ln_tile(ap_, i, h2T, g2c, b2cc)

        # ---------------- MLP + final residual (weights loaded once) ---------
        with tc.tile_pool(name="mlpp", bufs=1) as mp:
            m1g = mp.tile([P, FT, S], BF16, name="m1g")

            def m1_chunk(w1_sb, w, ftl, sc):
                ft = 4 * w + ftl
                ps = psum.tile([P, SC], F32, tag="pj", bufs=2, name="psm1")
                for ek in range(ET):
                    nc.tensor.matmul(
                        ps,
                        w1_sb[:, ek, ftl * P : (ftl + 1) * P],
                        h2T[sc][:, ek, :],
                        start=(ek == 0),
                        stop=(ek == ET - 1),
                    )
                nc.scalar.activation(
                    out=m1g[:, ft, sc * SC : (sc + 1) * SC],
                    in_=ps,
                    func=AF.Gelu,
                    bias=b1c[:, ft : ft + 1],
                    scale=1.0,
                )

            for i in range(ST):
                nc.gpsimd.tensor_add(x_hs[i], x_hs[i], b2b)
            # prefetch all w2 chunks early on the gpsimd queue
            w2_cs = []
            for c2 in range(E // MC):
                w2_c = mp.tile([P, FT, MC], BF16, tag="w2", bufs=3, name="w2_c")
                nc.gpsimd.dma_start(
                    w2_c, w2_t.ap()[c2].rearrange("p (k m) -> p k m", m=MC)
                )
                w2_cs.append(w2_c)
            for w in range(6):  # w1 column groups of 4 f-tiles
                w1_sb = mp.tile([P, ET, 512], BF16, tag="w1s", bufs=3,
                                name="w1_sb")
                nc.sync.dma_start(
                    w1_sb, w1_t.ap()[w].rearrange("p (k m) -> p k m", m=512)
                )
                for ftl in range(4):
                    for sc in range(NSC):
                        m1_chunk(w1_sb, w, ftl, sc)
            for c2 in range(E // MC):
                cs = slice(c2 * MC, (c2 + 1) * MC)
                for i in range(ST):
                    ps2 = psum.tile([P, SC], F32, tag="sc", bufs=2, name="psm2")
                    for fk in range(FT):
                        nc.tensor.matmul(
                            ps2[:, :MC],
                            m1g[:, fk, i * P : (i + 1) * P],
                            w2_cs[c2][:, fk, :],
                            start=(fk == 0),
                            stop=(fk == FT - 1),
                        )
                    otile = mp.tile([P, MC], F32, tag="otile", bufs=4,
                                    name="otile")
                    nc.vector.tensor_tensor(out=otile, in0=ps2[:, :MC],
                                            in1=x_hs[i][:, cs], op=ALU.add)
                    nc.sync.dma_start(outa[:, i, cs], otile)

    nc.compile()
    return nc


_CACHE = {}


def _get_nc(cfg):
    if cfg not in _CACHE:
        _CACHE[cfg] = build(cfg)
    return _CACHE[cfg]


def _prepare(inputs):
    """Host-side prep: classify score blocks, build mask pattern slabs, cast
    weights to bf16, and return (nc, in_maps) for run_bass_kernel_spmd."""
    inp = {
        k: np.ascontiguousarray(np.asarray(v, np.float32))
        for k, v in inputs.items()
    }
    mask = inp["mask"]  # [B, 1, S, S]

    blocks = []
    pat_key_to_slot = {}
    pats = []  # list of [B, P, SC] arrays
    for c in range(NSC):
        for tk in range(ST):
            blk = mask[:, 0, c * SC : (c + 1) * SC, tk * P : (tk + 1) * P]
            live = blk > -1e8  # [B, SC, P]
            if not live.any():
                continue  # fully masked for every batch
            if (blk == 0).all():
                blocks.append((tk, c, "clean", 0, 0))
                continue
            qlive = int(np.argmax(live.any(axis=(0, 2))))
            r = min(qlive, SC - 256)
            patT = np.maximum(blk, MASK_CLAMP).transpose(0, 2, 1)  # [B, P, SC]
            key = patT.tobytes()
            if key not in pat_key_to_slot:
                pat_key_to_slot[key] = len(pats)
                pats.append(patT)
            blocks.append((tk, c, "cross", pat_key_to_slot[key], r))
    n_pat = len(pats)
    if n_pat:
        maskpat = np.ascontiguousarray(
            np.stack(pats, axis=1).astype(ml_dtypes.bfloat16)
        )  # [B, n_pat, P, SC]
    else:
        maskpat = np.zeros((B, 1, P, SC), ml_dtypes.bfloat16)

    cfg = (tuple(blocks), n_pat)
    nc = _get_nc(cfg)

    shared = {k: inp[k] for k in ["bv", "bo", "b2"]}
    pcol = lambda v, n: inp[v].reshape(n, P).T
    shared["bias_pack"] = np.ascontiguousarray(
        np.concatenate(
            [pcol("bq", 6), pcol("bk", 6), pcol("ln1_g", 6), pcol("ln1_b", 6),
             pcol("ln2_g", 6), pcol("ln2_b", 6), pcol("b1", 24)],
            axis=1,
        ),
        np.float32,
    )
    # pre-tile weights (partition-major, chunk-contiguous) and cast to bf16 so
    # every DMA reads multi-KB contiguous lines per partition
    bf = lambda a: np.ascontiguousarray(a.astype(ml_dtypes.bfloat16))
    shared["wq"] = bf(inp["wq"].reshape(ET, P, E).transpose(1, 0, 2)
                      .reshape(P, ET * E))
    shared["wk"] = bf(inp["wk"].reshape(ET, P, E).transpose(1, 0, 2)
                      .reshape(P, ET * E))
    shared["wv"] = bf(inp["wv"].reshape(ET, P, E).transpose(1, 0, 2)
                      .reshape(P, ET * E))
    shared["wo"] = bf(inp["wo"].reshape(ET, P, 2, 384).transpose(2, 1, 0, 3)
                      .reshape(2, P, ET * 384))
    shared["w1"] = bf(inp["w1"].reshape(ET, P, 6, 512).transpose(2, 1, 0, 3)
                      .reshape(6, P, ET * 512))
    shared["w2"] = bf(inp["w2"].reshape(FT, P, E // MC, MC)
                      .transpose(2, 1, 0, 3).reshape(E // MC, P, FT * MC))
    in_maps = [
        {"x": inp["x"][b], "maskpat": maskpat[b], **shared} for b in range(B)
    ]
    return nc, in_maps


def kernel(**inputs) -> np.ndarray:
    nc, in_maps = _prepare(inputs)
    res = run_bass_kernel_spmd(nc, in_maps, core_ids=list(range(B)))
    return np.stack([res.results[b]["out"] for b in range(B)], axis=0)


if __name__ == "__main__":
    # smoke build with the causal block pattern
    blocks = []
    for c in range(NSC):
        for tk in range(ST):
            lo, hi = tk * P, (tk + 1) * P - 1  # key range
            qlo, qhi = c * SC, (c + 1) * SC - 1
            if lo > qhi:
                continue  # fully masked
            if hi <= qlo:
                blocks.append((tk, c, "clean", 0, 0))
            else:
                r = min(max(0, lo - qlo), SC - 256)
                blocks.append((tk, c, "cross", (lo - qlo) // P, r))
    build((tuple(blocks), 4))
    print("build ok")


# revision 29
# speedup vs baseline: 1.0105x; 1.0071x over previous
"""CLIP encoder layer (LN -> causal MHA -> residual -> LN -> GELU MLP -> residual)
as a Bass/Tile kernel for Trainium2, data-parallel over batch across 8 NeuronCores.

Layout strategy per core (one batch element):
  - matmul operands in bf16 (weights cast on the host; activations cast on the
    engine that moves them out of PSUM); all accumulation in fp32 PSUM; the
    residual stream x_h stays fp32 in SBUF. bf16 keeps the PE at 1 cycle/row
    with fast (FWL) weight loads and halves weight DMA + SBUF.
  - h kept natural [S,E] fp32 (residuals + LN stats) and transposed bf16 [E,S]
    (PE transposes; projections contract over E on partitions).
  - attention in scoresT[t_key, s_query] layout; softmax denominators come from
    a ones column appended to V (row 64 of the AV psum accumulation), kept in
    fp32(r) and inverted with the fast approximate reciprocal.
  - the additive mask is preprocessed on the HOST: per kept [128k x 512q] score
    block, transposed + clamped to -80, deduplicated into a few pattern slabs.
    All-zero (below-diagonal) blocks skip the mask add (exp reads score psum
    directly); fully-masked blocks are skipped; mask-dead leading query columns
    are skipped via live windows (floored at 256 wide). Per (head, chunk) the
    exp'd scores live in one contiguous strip so crossing blocks share one
    batched exp instruction.
  - MLP weights are loaded once; the full gelu map (bf16) stays in SBUF.
"""

import numpy as np
from contextlib import ExitStack

import ml_dtypes
import concourse.bass as bass
import concourse.mybir as mybir
import concourse.tile as tile
from concourse import bacc
from concourse.bass_utils import run_bass_kernel_spmd
from concourse.masks import make_identity

AF = mybir.ActivationFunctionType
ALU = mybir.AluOpType
F32 = mybir.dt.float32
F32R = mybir.dt.float32r
BF16 = mybir.dt.bfloat16

B, S, E, H, D, F = 8, 1024, 768, 12, 64, 3072
P = 128
ST = S // P            # 8 token tiles
ET = E // P            # 6 embed tiles
FT = F // P            # 24 ffn tiles
SC = 512               # attention s(query)-chunk
NSC = S // SC          # 2
NG = 3                 # head groups
HPG = H // NG          # 4 heads per group
GW = HPG * D           # 256 embed cols per group
MC = 256               # m2 output col chunk
SCALE = float(D) ** -0.5
EPS = 1e-5
MASK_CLAMP = -80.0     # exp(score - 80) == 0-ish for masked keys


def _r(ap):
    """Reinterpret an fp32 AP as float32r (used for the denominator path)."""
    return ap.bitcast(F32R)


def _bcast_dma(nc, dst, src_ap):
    """DMA a 1-D DRAM vector to [P, n] SBUF, broadcast along partitions."""
    bsrc = bass.AP(
        tensor=src_ap.tensor, offset=src_ap.offset, ap=[[0, P]] + list(src_ap.ap)
    )
    nc.gpsimd.dma_start(out=dst, in_=bsrc)


def build(cfg):
    """Build the Bass module.

    cfg = (blocks, n_pat); blocks is a tuple of (tk, c, kind, slot, r) with
    kind in {"clean", "cross"}; slot indexes the host-prepared maskT pattern
    slab; r is the live-window start column (floored so width >= 256).
    """
    blocks, n_pat = cfg
    nc = bacc.Bacc("TRN2", target_bir_lowering=False, debug=False, num_devices=8)

    x_t = nc.dram_tensor("x", [S, E], F32, kind="ExternalInput")
    mp_t = nc.dram_tensor("maskpat", [max(n_pat, 1), P, SC], BF16,
                          kind="ExternalInput")
    names_1d = ["bv", "bo", "b2"]
    v1 = {n: nc.dram_tensor(n, [E], F32, kind="ExternalInput") for n in names_1d}
    # [P, 60] host-packed per-channel vectors:
    # bq(6) bk(6) ln1g(6) ln1b(6) ln2g(6) ln2b(6) b1(24)
    bp_t = nc.dram_tensor("bias_pack", [P, 60], F32, kind="ExternalInput")
    # weights arrive host-pre-tiled, partition-major with long contiguous
    # per-partition lines so DMA runs at full bandwidth
    wq_t = nc.dram_tensor("wq", [P, ET * E], BF16, kind="ExternalInput")
    wk_t = nc.dram_tensor("wk", [P, ET * E], BF16, kind="ExternalInput")
    wv_t = nc.dram_tensor("wv", [P, ET * E], BF16, kind="ExternalInput")
    wo_t = nc.dram_tensor("wo", [2, P, ET * 384], BF16, kind="ExternalInput")
    w1_t = nc.dram_tensor("w1", [6, P, ET * 512], BF16, kind="ExternalInput")
    w2_t = nc.dram_tensor("w2", [E // MC, P, FT * MC], BF16,
                          kind="ExternalInput")
    out_t = nc.dram_tensor("out", [S, E], F32, kind="ExternalOutput")

    xa = x_t.ap().rearrange("(n p) e -> p n e", p=P)          # [P, ST, E]
    outa = out_t.ap().rearrange("(n p) e -> p n e", p=P)
    mpa = mp_t.ap().rearrange("n p k -> p n k")               # [P, n_pat, SC]

    binfo = {(tk, c): (kind, slot, r) for (tk, c, kind, slot, r) in blocks}
    # per chunk: clean blocks first (full width -> first AV matmul covers the
    # whole psum bank), then crossing blocks by ascending live-window r so the
    # crossing region of the exp strip is contiguous at the end.
    order = {}
    for c in range(NSC):
        ks = [(tk,) + binfo[(tk, c)] for tk in range(ST) if (tk, c) in binfo]
        order[c] = sorted(ks, key=lambda t: (t[1] != "clean", t[3], t[0]))

    with tile.TileContext(nc) as tc, ExitStack() as top:
        persist = top.enter_context(tc.tile_pool(name="persist", bufs=1))
        psum = top.enter_context(tc.tile_pool(name="psum", bufs=1, space="PSUM"))

        x_hs = [persist.tile([P, E], F32, name=f"x_h{i}") for i in range(ST)]
        identity = persist.tile([P, P], BF16, name="identity")
        make_identity(nc, identity)
        small = {"eps": persist.tile([P, 1], F32, name="eps_t")}
        nc.vector.memset(small["eps"], EPS)
        ones64 = persist.tile([1, 64], F32, name="ones64")
        nc.vector.memset(ones64, 1.0)
        bpk = persist.tile([P, 60], F32, name="bpk")
        bo_b = persist.tile([P, E], F32, name="bo_b")
        b2b = persist.tile([P, E], F32, name="b2b")
        bqs, bks = bpk[:, 0:6], bpk[:, 6:12]
        g1c, b1cc = bpk[:, 12:18], bpk[:, 18:24]
        g2c, b2cc = bpk[:, 24:30], bpk[:, 30:36]
        b1c = bpk[:, 36:60]

        nc.gpsimd.dma_start(bpk, bp_t.ap())
        for i in range(ST):
            nc.gpsimd.dma_start(x_hs[i], xa[:, i, :])

        h1T = [persist.tile([P, ET, SC], BF16, tag="hT", bufs=2, name="h1T")
               for _ in range(NSC)]

        def layernorm(pool, x_slice, out_tmp, seng=None):
            seng = seng or nc.vector
            xr = x_slice.rearrange("p (n s) -> p n s", s=256)
            stats = pool.tile([P, 3, 6], F32, tag="lnstats", bufs=4, name="st")
            for sg in range(3):
                seng.bn_stats(out=stats[:, sg, :], in_=xr[:, sg, :])
            mv = pool.tile([P, 2], F32, tag="lnmv", bufs=4, name="mv")
            seng.bn_aggr(out=mv, in_=stats)
            std = pool.tile([P, 2], F32, tag="lnrstd", bufs=4, name="std")
            nc.scalar.activation(
                out=std[:, 0:1], in_=mv[:, 1:2], func=AF.Sqrt,
                bias=small["eps"], scale=1.0
            )
            nc.vector.reciprocal_approx_fast(out=std[:, 1:2], in_=std[:, 0:1])
            nc.vector.tensor_scalar(
                out=out_tmp,
                in0=x_slice,
                scalar1=mv[:, 0:1],
                scalar2=std[:, 1:2],
                op0=ALU.subtract,
                op1=ALU.mult,
            )

        def ln_tile(pool, i, dstT, gc, bc, on_act=True, seng=None):
            htmp = pool.tile([P, E], BF16, tag="lntmp", bufs=3, name="htmp")
            layernorm(pool, x_hs[i], htmp, seng=seng)
            for j in range(ET):
                pt = psum.tile([P, P], BF16, tag="tp", bufs=2, name="pt")
                nc.tensor.transpose(pt, htmp[:, j * P : (j + 1) * P], identity)
                dst = dstT[i // 4][:, j, (i % 4) * P : (i % 4 + 1) * P]
                if on_act:
                    nc.scalar.activation(
                        out=dst,
                        in_=pt,
                        func=AF.Identity,
                        bias=bc[:, j : j + 1],
                        scale=gc[:, j : j + 1],
                    )
                else:
                    nc.vector.tensor_scalar(
                        out=dst,
                        in0=pt,
                        scalar1=gc[:, j : j + 1],
                        scalar2=bc[:, j : j + 1],
                        op0=ALU.mult,
                        op1=ALU.add,
                    )

        with tc.tile_pool(name="attnp", bufs=1) as ap_:
            maskpat = ap_.tile([P, max(n_pat, 1), SC], BF16, name="maskpat")
            bv_b = ap_.tile([P, E], F32, name="bv_b")
            OT = ap_.tile([P, ET, S], BF16, name="OT")

            wall = {}
            for nm, t in (("q", wq_t), ("k", wk_t), ("v", wv_t)):
                w = ap_.tile([P, ET, E], BF16, tag="wsl", bufs=3, name="w_all")
                nc.sync.dma_start(w, t.ap().rearrange("p (k m) -> p k m", m=E))
                wall[nm] = w
            wsl = {
                g: tuple(
                    wall[nm][:, :, g * GW : (g + 1) * GW] for nm in "qkv"
                )
                for g in range(NG)
            }
            wo_c = [
                ap_.tile([P, ET, 384], BF16, tag="wo", bufs=2, name="wo_c")
                for _ in range(2)
            ]

            def qk_proj(g, c):
                wq_g, wk_g, _ = wsl[g]
                qT_g, kT_g = qkT[g]
                for w_g, dstT, bias, scl in (
                    (wq_g, qT_g, bqs, SCALE),
                    (wk_g, kT_g, bks, None),
                ):
                    for jl in range(2):
                        jj = 2 * g + jl
                        ps = psum.tile([P, SC], F32, tag="pj", bufs=2, name="psq")
                        for ek in range(ET):
                            nc.tensor.matmul(
                                ps,
                                w_g[:, ek, jl * P : (jl + 1) * P],
                                h1T[c][:, ek, :],
                                start=(ek == 0),
                                stop=(ek == ET - 1),
                            )
                        if scl is None:
                            nc.vector.tensor_scalar(
                                out=dstT[:, jl, c * SC : (c + 1) * SC],
                                in0=ps,
                                scalar1=bias[:, jj : jj + 1],
                                scalar2=None,
                                op0=ALU.add,
                            )
                        else:
                            nc.vector.tensor_scalar(
                                out=dstT[:, jl, c * SC : (c + 1) * SC],
                                in0=ps,
                                scalar1=bias[:, jj : jj + 1],
                                scalar2=scl,
                                op0=ALU.add,
                                op1=ALU.mult,
                            )

            # LN1 interleaved with group-0 q/k projections
            qkT = {
                g: (
                    ap_.tile([P, 2, S], BF16, tag="qT", bufs=2, name="qT_g"),
                    ap_.tile([P, 2, S], BF16, tag="kT", bufs=2, name="kT_g"),
                )
                for g in range(NG)
            }
            for i in range(4):
                ln_tile(ap_, i, h1T, g1c, b1cc)
            qk_proj(0, 0)
            for i in range(4, ST):
                ln_tile(ap_, i, h1T, g1c, b1cc)
            qk_proj(0, 1)
            # deferred gpsimd-ring DMAs (keeps LN1's ring barrier short)
            nc.gpsimd.dma_start(maskpat, mpa)
            _bcast_dma(nc, bv_b, v1["bv"].ap())
            _bcast_dma(nc, bo_b, v1["bo"].ap())
            _bcast_dma(nc, b2b, v1["b2"].ap())
            for c2 in range(2):
                nc.gpsimd.dma_start(
                    wo_c[c2], wo_t.ap()[c2].rearrange("p (k m) -> p k m", m=384)
                )
            for i in range(ST):  # h += bo early (runs on idle gpsimd)
                nc.gpsimd.tensor_add(x_hs[i], x_hs[i], bo_b)

            for g in range(NG):
                _, _, wv_g = wsl[g]
                qT_g, kT_g = qkT[g]
                vaug = ap_.tile(
                    [P, ST, HPG, D + 1], BF16, tag="vg", bufs=2, name="vaug"
                )
                nc.gpsimd.memset(vaug[:, :, :, D : D + 1], 1.0)
                bvr = bv_b[:, g * GW : (g + 1) * GW].rearrange(
                    "p (h d) -> p h d", d=D
                )
                for i in range(ST):
                    ps = psum.tile([P, SC], F32, tag="pj", bufs=2, name="psv")
                    for ek in range(ET):
                        nc.tensor.matmul(
                            ps[:, :GW],
                            h1T[i // 4][:, ek, (i % 4) * P : (i % 4 + 1) * P],
                            wv_g[:, ek, :],
                            start=(ek == 0),
                            stop=(ek == ET - 1),
                        )
                    nc.vector.tensor_tensor(
                        out=vaug[:, i, :, 0:D],
                        in0=ps[:, :GW].rearrange("p (h d) -> p h d", d=D),
                        in1=bvr,
                        op=ALU.add,
                    )

                # per-head attention: emit both chunks' scores/exp first,
                # then both AV/normalize passes (keeps PE fed while ACT exps);
                # halfway through, emit the next group's q/k projections so the
                # PE has dense work while ACT/DVE catch up
                for hl in range(HPG):
                    if hl == 2 and g + 1 < NG:
                        qk_proj(g + 1, 0)
                        qk_proj(g + 1, 1)
                    hg = g * HPG + hl
                    jl, roff = hl // 2, 64 * (hl % 2)
                    st_c = {}
                    for c in range(NSC):
                        blks = order[c]
                        nb = len(blks)
                        strip = ap_.tile([P, nb * SC], BF16, tag="exs", bufs=4,
                                         name="strip")
                        st_c[c] = strip
                        ncl = sum(1 for t in blks if t[1] == "clean")
                        for n, (tk, kind, slot, r) in enumerate(blks):
                            pss = psum.tile([P, SC], F32, tag="sc", bufs=2,
                                            name="pss")
                            nc.tensor.matmul(
                                pss[:, r:],
                                kT_g[roff : roff + 64, jl,
                                     tk * P : (tk + 1) * P],
                                qT_g[roff : roff + 64, jl,
                                     c * SC + r : (c + 1) * SC],
                                start=True,
                                stop=True,
                            )
                            sl = strip[:, n * SC + r : (n + 1) * SC]
                            if kind == "cross":
                                nc.vector.tensor_tensor(
                                    out=sl, in0=pss[:, r:],
                                    in1=maskpat[:, slot, r:], op=ALU.add,
                                )
                            else:
                                nc.scalar.activation(sl, pss[:, r:],
                                                     func=AF.Exp)
                        if ncl < nb:
                            reg = strip[:, ncl * SC : nb * SC]
                            nc.scalar.activation(reg, reg, func=AF.Exp)
                    for c in range(NSC):
                        blks = order[c]
                        nb = len(blks)
                        strip = st_c[c]
                        psa = psum.tile([P, SC], F32, tag="av", bufs=2,
                                        name="psa")
                        for n, (tk, kind, slot, r) in enumerate(blks):
                            nc.tensor.matmul(
                                psa[: D + 1, r:],
                                vaug[:, tk, hl, :],
                                strip[:, n * SC + r : (n + 1) * SC],
                                start=(n == 0),
                                stop=(n == nb - 1),
                            )
                        denr = ap_.tile([1, SC], F32, tag="denr", bufs=2,
                                        name="denr")
                        nc.vector.tensor_copy(_r(denr), psa[D : D + 1, :])
                        psb = psum.tile([P, SC], F32, tag="tp", bufs=2,
                                        name="psb")
                        nc.tensor.matmul(
                            psb[:64, :], _r(ones64), _r(denr), start=True,
                            stop=True
                        )
                        bcs = ap_.tile([64, SC], F32, tag="bcs", bufs=2,
                                       name="bcs")
                        nc.vector.reciprocal_approx_fast(out=bcs,
                                                         in_=psb[:64, :])
                        ro2 = 64 * (hg % 2)
                        nc.vector.tensor_tensor(
                            out=OT[ro2 : ro2 + 64, hg // 2,
                                   c * SC : (c + 1) * SC],
                            in0=psa[:D, :],
                            in1=bcs,
                            op=ALU.mult,
                        )

            # ---------------- output projection + residual + LN2 -------------
            h2T = [persist.tile([P, ET, SC], BF16, tag="hT", bufs=2,
                                name="h2T") for _ in range(NSC)]
            m1g_w0 = persist.tile([P, 4, S], BF16, name="m1g_w0")
            w1_sb0 = persist.tile([P, ET, 512], BF16, name="w1_sb0")
            nc.sync.dma_start(
                w1_sb0, w1_t.ap()[0].rearrange("p (k m) -> p k m", m=512)
            )

            def m1_chunk(w1_sb, w, ftl, sc):
                ft = 4 * w + ftl
                ps = psum.tile([P, SC], F32, tag="pj", bufs=2, name="psm1")
                for ek in range(ET):
                    nc.tensor.matmul(
                        ps,
                        w1_sb[:, ek, ftl * P : (ftl + 1) * P],
                        h2T[sc][:, ek, :],
                        start=(ek == 0),
                        stop=(ek == ET - 1),
                    )
                dst = (m1g_w0[:, ft, sc * SC : (sc + 1) * SC] if w == 0
                       else m1g_rest[:, ft - 4, sc * SC : (sc + 1) * SC])
                nc.scalar.activation(
                    out=dst,
                    in_=ps,
                    func=AF.Gelu,
                    bias=b1c[:, ft : ft + 1],
                    scale=1.0,
                )

            for i in range(ST):
                if i == 6:
                    for ftl in range(4):  # early m1 on the ready sc=0 chunk
                        m1_chunk(w1_sb0, 0, ftl, 0)
                for c2 in range(2):
                    cs = slice(c2 * 384, (c2 + 1) * 384)
                    ps = psum.tile([P, SC], F32, tag="pj", bufs=2, name="pso")
                    for ek in range(ET):
                        nc.tensor.matmul(
                            ps[:, :384],
                            OT[:, ek, i * P : (i + 1) * P],
                            wo_c[c2][:, ek, :],
                            start=(ek == 0),
                            stop=(ek == ET - 1),
                        )
                    nc.vector.tensor_tensor(out=x_hs[i][:, cs], in0=ps[:, :384],
                                            in1=x_hs[i][:, cs], op=ALU.add)
                ln_tile(ap_, i, h2T, g2c, b2cc)

        # ---------------- MLP + final residual (weights loaded once) ---------
        with tc.tile_pool(name="mlpp", bufs=1) as mp:
            m1g_rest = mp.tile([P, FT - 4, S], BF16, name="m1g_rest")
            for i in range(ST):
                nc.gpsimd.tensor_add(x_hs[i], x_hs[i], b2b)
            for ftl in range(4):
                m1_chunk(w1_sb0, 0, ftl, 1)
            # prefetch all w2 chunks early on the gpsimd queue
            w2_cs = []
            for c2 in range(E // MC):
                w2_c = mp.tile([P, FT, MC], BF16, tag="w2", bufs=3, name="w2_c")
                nc.gpsimd.dma_start(
                    w2_c, w2_t.ap()[c2].rearrange("p (k m) -> p k m", m=MC)
                )
                w2_cs.append(w2_c)
            for w in range(1, 6):  # w1 column groups of 4 f-tiles
                w1_sb = mp.tile([P, ET, 512], BF16, tag="w1s", bufs=3,
                                name="w1_sb")
                nc.sync.dma_start(
                    w1_sb, w1_t.ap()[w].rearrange("p (k m) -> p k m", m=512)
                )
                for ftl in range(4):
                    for sc in range(NSC):
                        m1_chunk(w1_sb, w, ftl, sc)
            for c2 in range(E // MC):
                cs = slice(c2 * MC, (c2 + 1) * MC)
                for i in range(ST):
                    ps2 = psum.tile([P, SC], F32, tag="sc", bufs=2, name="psm2")
                    for fk in range(FT):
                        nc.tensor.matmul(
                            ps2[:, :MC],
                            (m1g_w0[:, fk, i * P : (i + 1) * P] if fk < 4
                             else m1g_rest[:, fk - 4, i * P : (i + 1) * P]),
                            w2_cs[c2][:, fk, :],
                            start=(fk == 0),
                            stop=(fk == FT - 1),
                        )
                    otile = mp.tile([P, MC], F32, tag="otile", bufs=4,
                                    name="otile")
                    nc.vector.tensor_tensor(out=otile, in0=ps2[:, :MC],
                                            in1=x_hs[i][:, cs], op=ALU.add)
                    nc.sync.dma_start(outa[:, i, cs], otile)

    nc.compile()
    return nc


_CACHE = {}


def _get_nc(cfg):
    if cfg not in _CACHE:
        _CACHE[cfg] = build(cfg)
    return _CACHE[cfg]


def _prepare(inputs):
    """Host-side prep: classify score blocks, build mask pattern slabs, cast
    weights to bf16, and return (nc, in_maps) for run_bass_kernel_spmd."""
    inp = {
        k: np.ascontiguousarray(np.asarray(v, np.float32))
        for k, v in inputs.items()
    }
    mask = inp["mask"]  # [B, 1, S, S]

    blocks = []
    pat_key_to_slot = {}
    pats = []  # list of [B, P, SC] arrays
    for c in range(NSC):
        for tk in range(ST):
            blk = mask[:, 0, c * SC : (c + 1) * SC, tk * P : (tk + 1) * P]
            live = blk > -1e8  # [B, SC, P]
            if not live.any():
                continue  # fully masked for every batch
            if (blk == 0).all():
                blocks.append((tk, c, "clean", 0, 0))
                continue
            qlive = int(np.argmax(live.any(axis=(0, 2))))
            r = min(qlive, SC - 256)
            patT = np.maximum(blk, MASK_CLAMP).transpose(0, 2, 1)  # [B, P, SC]
            key = patT.tobytes()
            if key not in pat_key_to_slot:
                pat_key_to_slot[key] = len(pats)
                pats.append(patT)
            blocks.append((tk, c, "cross", pat_key_to_slot[key], r))
    n_pat = len(pats)
    if n_pat:
        maskpat = np.ascontiguousarray(
            np.stack(pats, axis=1).astype(ml_dtypes.bfloat16)
        )  # [B, n_pat, P, SC]
    else:
        maskpat = np.zeros((B, 1, P, SC), ml_dtypes.bfloat16)

    cfg = (tuple(blocks), n_pat)
    nc = _get_nc(cfg)

    shared = {k: inp[k] for k in ["bv", "bo", "b2"]}
    pcol = lambda v, n: inp[v].reshape(n, P).T
    shared["bias_pack"] = np.ascontiguousarray(
        np.concatenate(
            [pcol("bq", 6), pcol("bk", 6), pcol("ln1_g", 6), pcol("ln1_b", 6),
             pcol("ln2_g", 6), pcol("ln2_b", 6), pcol("b1", 24)],
            axis=1,
        ),
        np.float32,
    )
    # pre-tile weights (partition-major, chunk-contiguous) and cast to bf16 so
    # every DMA reads multi-KB contiguous lines per partition
    bf = lambda a: np.ascontiguousarray(a.astype(ml_dtypes.bfloat16))
    shared["wq"] = bf(inp["wq"].reshape(ET, P, E).transpose(1, 0, 2)
                      .reshape(P, ET * E))
    shared["wk"] = bf(inp["wk"].reshape(ET, P, E).transpose(1, 0, 2)
                      .reshape(P, ET * E))
    shared["wv"] = bf(inp["wv"].reshape(ET, P, E).transpose(1, 0, 2)
                      .reshape(P, ET * E))
    shared["wo"] = bf(inp["wo"].reshape(ET, P, 2, 384).transpose(2, 1, 0, 3)
                      .reshape(2, P, ET * 384))
    shared["w1"] = bf(inp["w1"].reshape(ET, P, 6, 512).transpose(2, 1, 0, 3)
                      .reshape(6, P, ET * 512))
    shared["w2"] = bf(inp["w2"].reshape(FT, P, E // MC, MC)
                      .transpose(2, 1, 0, 3).reshape(E // MC, P, FT * MC))
    in_maps = [
        {"x": inp["x"][b], "maskpat": maskpat[b], **shared} for b in range(B)
    ]
    return nc, in_maps


def kernel(**inputs) -> np.ndarray:
    nc, in_maps = _prepare(inputs)
    res = run_bass_kernel_spmd(nc, in_maps, core_ids=list(range(B)))
    return np.stack([res.results[b]["out"] for b in range(B)], axis=0)


if __name__ == "__main__":
    # smoke build with the causal block pattern
    blocks = []
    for c in range(NSC):
        for tk in range(ST):
            lo, hi = tk * P, (tk + 1) * P - 1  # key range
            qlo, qhi = c * SC, (c + 1) * SC - 1
            if lo > qhi:
                continue  # fully masked
            if hi <= qlo:
                blocks.append((tk, c, "clean", 0, 0))
            else:
                r = min(max(0, lo - qlo), SC - 256)
                blocks.append((tk, c, "cross", (lo - qlo) // P, r))
    build((tuple(blocks), 4))
    print("build ok")
